# revision 13
# baseline (speedup 1.0000x reference)
"""Trainium2 Bass kernel for nn_Attention_32049045963483 (sparse_attention).

Math collapse (validated vs reference at ~4e-4 l2):
  - qkv 1x1 conv folds into the 11x11/stride-8 down-convs ON HOST:
      w_eff[o,d,ky,kx] = sum_ic wq[o,ic,ky,kx] w_qkv[ic,d]   (weights only)
  - nearest-neighbor 64x upsample of the score map + softmax over the
    upsampled axis == softmax of the low-res map; every output row depends
    only on x = row//64.
  - v enters only through 64-wide block sums: vbar = Wv @ fbar,
      fbar[d,J] = sum_y f[d,J,y]  (v never materializes)
  - out[c,x,y] = (sum_J e[J,x] vbar[c,J]) / (64 sum_J e[J,x]), e = exp of
    the scaled low-res dots, broadcast along y.
  - GELU via the tanh approximation computed as x*(1+tanh(x(c1+c2 x^2))),
    i.e. 2*gelu; the 0.5 factors from q and k fold into the exp scale
    (scale/4).  Square/Tanh/Exp all live in the ACT `exp_and_others` table
    set -> exactly ONE ACT table load, no mid-kernel table switch.

HW lessons baked in (measured on this part):
  - PE runs at a fixed ~1.2GHz here; column count is the invariant, so the
    stage-1 contraction packs TWO kx taps into 128 partitions (partitions
    64..127 hold a copy of f shifted by one tap, built on host): 6 paired
    matmuls per conv instead of 11.
  - A strided (16B-step) fp16 moving operand streams at ~half rate; f is
    stored deinterleaved as (xmod, oxslot, row) so each tap's rhs is a
    [128, 8, 64] AP with 128B-contiguous inner runs.
  - DMA descriptors >4096B hit a slow path (~13GB/s/engine vs ~26);
    every transfer is sliced to <=4KB per partition.

Sharding: head-parallel over 8 cores; core i computes channels 8i..8i+7.
The (ox,oy)-permuted position order used on-chip is undone on the host.
"""

import numpy as np

N_CORES = 8
SCALE4 = (8.0 ** -0.5) / 4.0  # dim_head**-0.5, /4 folds the two gelu 0.5s
GC1 = 0.7978845608028654      # sqrt(2/pi)
GC2 = GC1 * 0.044715

FW = 8 * 9 * 68               # 4896 fp16 els per partition of the f tile

_CACHE = {}

LAST_RESULTS = None  # BassKernelResults of the most recent run (for test harness)


def _build_nc():
    from contextlib import ExitStack

    import concourse.bacc as bacc
    import concourse.bass as bass
    import concourse.mybir as mybir
    import concourse.tile as tile

    f32 = mybir.dt.float32
    f16 = mybir.dt.float16
    X = mybir.AxisListType.X
    AF = mybir.ActivationFunctionType
    ALU = mybir.AluOpType

    nc = bacc.Bacc("TRN2", target_bir_lowering=False)

    f_d = nc.dram_tensor("f", [128, FW], f16, kind="ExternalInput")
    wm_d = nc.dram_tensor("wm", [128, 1160], f16, kind="ExternalInput")
    out_d = nc.dram_tensor("out", [64, 512], f32, kind="ExternalOutput")

    with tile.TileContext(nc) as tc:
        with ExitStack() as ctx:
            sb = ctx.enter_context(tc.tile_pool(name="sb", bufs=1))
            ps = ctx.enter_context(tc.tile_pool(name="ps", bufs=1, space="PSUM"))

            ftile = sb.tile([128, FW], f16)
            wm_t = sb.tile([128, 1160], f16)
            sq_t = sb.tile([89, 536], f16)
            sk_t = sb.tile([89, 536], f16)
            fbar_t = sb.tile([64, 64], f16)
            q_t = sb.tile([8, 64], f16)
            k_t = sb.tile([8, 64], f16)
            e_t = sb.tile([64, 64], f16)
            vaug_t = sb.tile([64, 9], f16)
            rs_t = sb.tile([64, 1], f32)
            T_t = sb.tile([64, 512], f32)
            # gelu scratch (separate q/k sets so the chains overlap freely)
            gsq_q = sb.tile([8, 64], f32)
            ga_q = sb.tile([8, 64], f32)
            gv_q = sb.tile([8, 64], f32)
            gt_q = sb.tile([8, 64], f16)
            gsq_k = sb.tile([8, 64], f32)
            ga_k = sb.tile([8, 64], f32)
            gv_k = sb.tile([8, 64], f32)
            gt_k = sb.tile([8, 64], f16)
            scr_t = sb.tile([1, 1], f32)
            scr2_t = sb.tile([1, 1], f32)

            # fbar add-tree scratch
            tb1 = sb.tile([64, 2304], f16)
            tb2 = sb.tile([64, 1152], f16)
            tb3 = sb.tile([64, 576], f16)

            # --- input DMAs FIRST (so descriptor emission isn't queued
            # behind the ACT table preload / DVE memsets on the sequencers).
            # Weights ride the scalar ring; f is sliced at xmod-block
            # boundaries (<=4KB per partition) on the sync ring so stage-1
            # taps in blocks 0..2 can start while later blocks stream.
            nc.sync.dma_start(out=wm_t, in_=wm_d[:])
            BLK = 612  # one xmod block, els
            for c0, c1 in ((0, 3 * BLK), (3 * BLK, 6 * BLK), (6 * BLK, 8 * BLK)):
                nc.sync.dma_start(out=ftile[:, c0:c1], in_=f_d[:, c0:c1])

            # sq/sk layout: [89, (ox 8, r 67)].  Ones-row 88 rides a [64:89]
            # memset (mod-32 start rule); borders r in {0,1,66} re-zeroed,
            # cast fills r=2..66 interior.
            sq3 = sq_t.rearrange("p (ox r) -> p ox r", r=67)
            sk3 = sk_t.rearrange("p (ox r) -> p ox r", r=67)
            nc.vector.memset(sq_t[64:89, :], 1.0)
            nc.vector.memset(sq3[0:88, :, 0:2], 0.0)
            nc.vector.memset(sq3[0:88, :, 66:67], 0.0)
            nc.vector.memset(sk_t[64:89, :], 1.0)
            nc.vector.memset(sk3[0:88, :, 0:2], 0.0)
            nc.vector.memset(sk3[0:88, :, 66:67], 0.0)
            nc.vector.memset(vaug_t[:, 8:9], 64.0)
            nc.vector.memset(scr_t, 0.0)

            # one ACT table load for the whole kernel (exp_and_others:
            # exp + tanh + square + copy).  Runs during the DMA wait.
            nc.scalar.activation(out=scr2_t, in_=scr_t, func=AF.Exp)

            # f view: [128, xmod 8, oxslot 9, r 68]
            f4 = ftile.rearrange("p (xm oxs r) -> p xm oxs r", xm=8, r=68)
            wmp = wm_t[:, 0:1056].rearrange("p (cv j m) -> p cv j m", cv=2, m=88)
            wvt_v = wm_t[0:64, 1152:1160]

            # --- stage 1: 6 paired kx-taps accumulate; rhs [128, ox 8, r 64]
            # (partitions 64..127 = f shifted one tap); free = 512.  Pair
            # order follows DMA slice arrival: taps in xm blocks 0..2 first.
            psq = ps.tile([88, 512], f32)
            psk = ps.tile([88, 512], f32)
            psq3 = psq.rearrange("p (ox r) -> p ox r", r=64)
            psk3 = psk.rearrange("p (ox r) -> p ox r", r=64)
            PAIR_ORDER = (0, 1, 4, 5, 2, 3)  # xm blocks 0,2,0,2,4,6

            def s1(ps3, cv):
                for n, j in enumerate(PAIR_ORDER):
                    kx = 2 * j
                    nc.tensor.matmul(
                        ps3, wmp[:, cv, j],
                        f4[:, kx % 8, kx // 8 : kx // 8 + 8, 2:66],
                        start=(n == 0), stop=(n == 5),
                    )

            s1(psq3, 0)
            s1(psk3, 1)

            # --- fbar[d, J] = sum_x f16[d, row 2+J, x] via a contiguous
            # add-tree over xmod halves (the one 4D strided reduce was ~4x
            # slower and blocked the casts), interleaved so cast_q runs
            # between tree stages.
            fh = ftile.rearrange("p (h b r) -> p h b r", h=2, r=68)
            t1 = tb1.rearrange("p (b r) -> p b r", r=64)   # b = (xm 0..3, oxs)
            t2 = tb2.rearrange("p (b r) -> p b r", r=64)
            t3 = tb3.rearrange("p (b r) -> p b r", r=64)
            with nc.allow_low_precision(reason="fp16 staged sums"):
                nc.vector.tensor_tensor(
                    out=t1, in0=fh[0:64, 0, :, 2:66], in1=fh[0:64, 1, :, 2:66],
                    op=ALU.add,
                )
                cast_q = nc.vector.tensor_copy(out=sq3[0:88, :, 2:66], in_=psq3)
                cast_k = nc.vector.tensor_copy(out=sk3[0:88, :, 2:66], in_=psk3)
                nc.vector.tensor_tensor(
                    out=t2, in0=t1[:, 0:18], in1=t1[:, 18:36], op=ALU.add
                )
                nc.vector.tensor_tensor(
                    out=t3, in0=t2[:, 0:9], in1=t2[:, 9:18], op=ALU.add
                )
                nc.vector.reduce_sum(
                    out=fbar_t, in_=t3.transpose([0, 2, 1]), axis=X
                )

            # --- stage 2 + bias (ones-row 88 against the bias row of the
            # ky=0 stationary); free = (ox, oy) -> permuted position order
            psc_q = ps.tile([8, 64], f32)
            psc_k = ps.tile([8, 64], f32)

            def s2(psc, s3, aug_col):
                nc.tensor.matmul(
                    psc, wm_t[0:89, aug_col : aug_col + 8], s3[0:89, :, 0:57:8],
                    start=True, stop=False,
                )
                for ky in range(1, 11):
                    nc.tensor.matmul(
                        psc, wm_t[0:88, 1064 + 8 * ky : 1072 + 8 * ky],
                        s3[0:88, :, ky : ky + 57 : 8],
                        start=False, stop=(ky == 10),
                    )

            s2(psc_q, sq3, 1056)
            s2(psc_k, sk3, 1064)

            # --- gelu (x2) via tanh approx: g2 = x*(1+tanh(x*(c1+c2*x^2)))
            # The final write permutes (ox,oy)->position order oy*8+ox via a
            # 2-free-dim out AP, so everything downstream is in natural
            # spatial order (stationary matmul APs must stay 2D).
            def gelu2(psc, gsq, ga, gv, gt, gout):
                nc.scalar.activation(out=gsq, in_=psc, func=AF.Square)
                # gv = (x^2 + c1/c2) * x; the c2 factor folds into tanh scale
                nc.vector.scalar_tensor_tensor(
                    out=gv, in0=gsq, scalar=GC1 / GC2, in1=psc,
                    op0=ALU.add, op1=ALU.mult,
                )
                nc.scalar.activation(out=gt, in_=gv, func=AF.Tanh, scale=GC2)
                gout_p = gout.rearrange("p (oy ox) -> p oy ox", ox=8).transpose(
                    [0, 2, 1]
                )
                gt3 = gt.rearrange("p (ox oy) -> p ox oy", oy=8)
                psc3 = psc.rearrange("p (ox oy) -> p ox oy", oy=8)
                nc.vector.scalar_tensor_tensor(
                    out=gout_p, in0=gt3, scalar=1.0, in1=psc3,
                    op0=ALU.add, op1=ALU.mult,
                )

            gelu2(psc_q, gsq_q, ga_q, gv_q, gt_q, q_t)
            gelu2(psc_k, gsq_k, ga_k, gv_k, gt_k, k_t)

            # --- vbar path: psv[J, c] = sum_d fbar[d, J] wvt[d, c]
            psv = ps.tile([64, 8], f32)
            nc.tensor.matmul(psv, fbar_t, wvt_v, start=True, stop=True)
            nc.vector.tensor_copy(out=vaug_t[:, 0:8], in_=psv)

            # --- dots_T[J, I] = sum_c k2[c, J] q2[c, I]; e = exp(scale/4)
            psd = ps.tile([64, 64], f32)
            nc.tensor.matmul(psd, k_t, q_t, start=True, stop=True)
            nc.scalar.activation(out=e_t, in_=psd, func=AF.Exp, scale=SCALE4)

            # --- out_u[I', 0:8] = sum_J e[J', I'] vbar[J', c]; col 8 = 64*sum e
            pso = ps.tile([64, 9], f32)
            nc.tensor.matmul(pso, e_t, vaug_t, start=True, stop=True)
            nc.vector.reciprocal(out=rs_t, in_=pso[:, 8:9])

            # --- T[x', c, y] = pso[x', c] * rs[x']  (stride-0 broadcast on y)
            T3 = T_t.rearrange("p (c y) -> p c y", y=64)
            p0 = pso[:, 0:8]
            in0 = bass.AP(
                tensor=p0.tensor, offset=p0.offset,
                ap=[list(p0.ap[0]), list(p0.ap[1]), [0, 64]],
            )
            r0 = rs_t[:]
            in1 = bass.AP(
                tensor=r0.tensor, offset=r0.offset,
                ap=[list(r0.ap[0]), [0, 8], [0, 64]],
            )
            nc.vector.tensor_tensor(out=T3, in0=in0, in1=in1, op=ALU.mult)

            # --- store: out_d[x', (c y)] <- T; host unpermutes x' and
            # transposes to [c, x, y]
            nc.scalar.dma_start(out=out_d[:], in_=T_t[:])

    nc.finalize()
    return nc


def _get_nc():
    if "nc" not in _CACHE:
        _CACHE["nc"] = _build_nc()
    return _CACHE["nc"]


def _prep_inputs(inputs):
    f = np.ascontiguousarray(inputs["f"], np.float32)
    w_qkv = np.ascontiguousarray(inputs["w_qkv"], np.float32)[:, :, 0, 0]  # [192,64]
    wq = np.ascontiguousarray(inputs["wq"], np.float32)
    wk = np.ascontiguousarray(inputs["wk"], np.float32)
    bq = np.ascontiguousarray(inputs["bq"], np.float32)
    bk = np.ascontiguousarray(inputs["bk"], np.float32)

    # deinterleaved, padded f: fdei[d, xm, oxs, r] = fpad[d, r, 8*oxs + xm]
    fpad = np.zeros((64, 68, 68), np.float32)
    fpad[:, 2:66, 2:66] = f[0]
    fdei = np.zeros((64, 8, 9, 68), np.float16)
    for xm in range(8):
        w = fpad[:, :, xm::8]  # [64, 68 r, ncols], x = xm + 8*oxs
        fdei[:, xm, : w.shape[2], :] = w.transpose(0, 2, 1)
    # partitions 64..127: shifted one tap (one xmod block)
    fd = np.zeros((128, 8, 9, 68), np.float16)
    fd[0:64] = fdei
    fd[64:128, 0:7] = fdei[:, 1:8]
    f_flat = fd.reshape(128, FW)

    w1q, w1k = w_qkv[0:64], w_qkv[64:128]
    in_maps = []
    for i in range(N_CORES):
        sl = slice(8 * i, 8 * i + 8)
        # w_eff[o,d,ky,kx] -> per-tap [d, ky*8+o]
        weq = np.einsum("oikl,id->odkl", wq[sl], w1q).astype(np.float16)
        wek = np.einsum("oikl,id->odkl", wk[sl], w1k).astype(np.float16)
        wm = np.zeros((128, 1160), np.float16)
        for cv, we in ((0, weq), (1, wek)):
            for j in range(6):
                blk = slice(528 * cv + 88 * j, 528 * cv + 88 * j + 88)
                # [o,d,ky] for kx=2j -> [d, (ky,o)]
                wm[0:64, blk] = we[:, :, :, 2 * j].transpose(1, 2, 0).reshape(64, 88)
                if 2 * j + 1 < 11:
                    wm[64:128, blk] = (
                        we[:, :, :, 2 * j + 1].transpose(1, 2, 0).reshape(64, 88)
                    )
        # stage-2 selection + bias rows: cols 1056:1064 q-aug(ky=0),
        # 1064:1072 k-aug(ky=0), 1072:1152 sel(ky=1..10)
        for o in range(8):
            wm[o, 1056 + o] = 1.0
            wm[o, 1064 + o] = 1.0
        wm[88, 1056:1064] = bq[sl]
        wm[88, 1064:1072] = bk[sl]
        for ky in range(1, 11):
            for o in range(8):
                wm[ky * 8 + o, 1064 + 8 * ky + o] = 1.0
        wm[0:64, 1152:1160] = w_qkv[128 + 8 * i : 136 + 8 * i].T
        in_maps.append({"f": f_flat, "wm": wm})
    return in_maps


def _unshard(results):
    # per-core out [64(x), 512(c,y)] -> full [1, 64, 64, 64]
    out = np.empty((64, 64, 64), np.float32)
    for i, r in enumerate(results):
        t = r["out"].reshape(64, 8, 64)  # [x, c, y]
        out[8 * i : 8 * i + 8] = t.transpose(1, 0, 2)
    return out.reshape(1, 64, 64, 64)


def kernel(**inputs):
    global LAST_RESULTS
    from concourse.bass_utils import run_bass_kernel_spmd

    in_maps = _prep_inputs(inputs)
    nc = _get_nc()
    res = run_bass_kernel_spmd(nc, in_maps, core_ids=list(range(N_CORES)))
    LAST_RESULTS = res
    return _unshard(res.results)
